# revision 1
# baseline (speedup 1.0000x reference)
"""Trainium2 Bass kernel for a 2-layer GAT (CeleTrip word-doc graph).

Contract: kernel(**inputs) takes the full unsharded inputs of
reference.setup_inputs() and returns the full [1, 2] float32 output.

Strategy (8 NeuronCores):
 - Node permutation pi balances doc/word rows per core; core c owns pi-nodes
   [c*NPC, (c+1)*NPC) as edge-softmax destinations.
 - Dense projections sharded by node; per layer the per-node table
   [h(256 interleaved) | el(4)] (bf16, 768B rows) is AllGather'd so every
   core can gather arbitrary source rows with dma_gather (int16 indices ->
   table split in lo/hi halves at row 32768).
 - Edge phase: edges sorted by dst block (128 dst per PSUM tile); per
   128-edge chunk a one-hot mask (dst-local vs iota) is built on DVE and two
   PE matmuls accumulate the p-weighted features and the softmax denominators
   into PSUM.  p = exp(leaky_relu(el_src+er_dst)) = max(exp(L), exp(0.2 L)).
 - er rows are gathered from a per-core local table (256B rows).
 - Final graph max-pool: per-core partition-wise running max -> host max +
   tiny host MLP (0.03% of FLOPs).
"""

import math
import os
from contextlib import ExitStack
from dataclasses import dataclass

import numpy as np
import ml_dtypes

import concourse.bass as bass
import concourse.mybir as mybir
import concourse.tile as tile
import concourse.bacc as bacc
from concourse.bass_utils import run_bass_kernel_spmd

bf16 = ml_dtypes.bfloat16
F32 = mybir.dt.float32
BF16 = mybir.dt.bfloat16
I16 = mybir.dt.int16

H = 4
D = 64
F = H * D            # 256
ROW = 384            # table row (bf16 elems): 256 h + 4 el + pad -> 768B
ERROW = 128          # er table row (bf16 elems) -> 256B


@dataclass
class Cfg:
    n_doc: int = 5000
    n_word: int = 45000
    n_cores: int = 8
    fd_doc: int = 768
    fd_word: int = 300
    losplit: int = 32768

    @property
    def n(self):
        return self.n_doc + self.n_word

    @property
    def npc(self):
        return self.n // self.n_cores

    @property
    def doc_pc(self):
        return self.n_doc // self.n_cores

    @property
    def word_pc(self):
        return self.n_word // self.n_cores

    @property
    def nblk(self):
        return (self.npc + 127) // 128

    @property
    def npad(self):
        return self.nblk * 128

    @property
    def fd_word_pad(self):
        return ((self.fd_word + 127) // 128) * 128


# interleave permutation: position p in interleaved layout holds natural
# feature (p % 4) * 64 + p // 4   (i.e. [j, h] order instead of [h, j])
NAT_OF_POS = (np.arange(F) % H) * D + np.arange(F) // H     # ilv <- nat
POS_OF_NAT = np.argsort(NAT_OF_POS)                          # nat -> ilv


def _f32(x):
    return np.ascontiguousarray(np.asarray(x), dtype=np.float32)


def _perm_old2new(cfg: Cfg):
    """old node id -> pi node id."""
    p = np.empty(cfg.n, np.int64)
    d = np.arange(cfg.n_doc)
    p[d] = (d // cfg.doc_pc) * cfg.npc + d % cfg.doc_pc
    w = np.arange(cfg.n_word)
    p[cfg.n_doc + w] = (w // cfg.word_pc) * cfg.npc + cfg.doc_pc + w % cfg.word_pc
    return p


def _host_prep(cfg: Cfg, inputs):
    """Build per-core device input maps + build-time metadata."""
    doc_feat = _f32(inputs["doc_feat"])
    word_feat = _f32(inputs["word_feat"])
    src = np.asarray(inputs["src"]).astype(np.int64)
    dst = np.asarray(inputs["dst"]).astype(np.int64)

    Wd = _f32(inputs["Wd"]); bd = _f32(inputs["bd"])
    Ww = _f32(inputs["Ww"]); bw = _f32(inputs["bw"])
    W1 = _f32(inputs["W1"]); al1 = _f32(inputs["al1"]); ar1 = _f32(inputs["ar1"])
    b1 = _f32(inputs["b1"]); resW1 = _f32(inputs["resW1"])
    W2 = _f32(inputs["W2"]); al2 = _f32(inputs["al2"]); ar2 = _f32(inputs["ar2"])
    b2 = _f32(inputs["b2"])

    perm = _perm_old2new(cfg)
    s = perm[src]
    d = perm[dst]

    # ---- per-core edge structures --------------------------------------
    owner = d // cfg.npc
    NCOR = cfg.n_cores
    per_core = []
    for c in range(NCOR):
        m = owner == c
        sc = s[m]
        dloc = d[m] - c * cfg.npc
        blk = dloc // 128
        bloc = dloc % 128
        hi = (sc >= cfg.losplit).astype(np.int64)
        # sort by (blk, hi); order within groups irrelevant
        order = np.lexsort((hi, blk))
        per_core.append((sc[order], dloc[order], blk[order], bloc[order], hi[order]))

    nblk = cfg.nblk
    # per-core per-block lo/hi counts
    nlo_e = np.zeros((NCOR, nblk), np.int64)
    nhi_e = np.zeros((NCOR, nblk), np.int64)
    for c in range(NCOR):
        sc, dloc, blk, bloc, hi = per_core[c]
        for b in range(nblk):
            mb = blk == b
            nhi_e[c, b] = int((hi[mb] == 1).sum())
            nlo_e[c, b] = int(mb.sum()) - nhi_e[c, b]
    # fake self-edges for padded dst slots in the last block (avoid s==0)
    pad_dst = cfg.npad - cfg.npc
    nlo_e[:, nblk - 1] += pad_dst

    # chunk counts, maxed across cores so every core runs the same program
    nloC = np.maximum(1, (nlo_e.max(axis=0) + 127) // 128).astype(np.int64)
    nhiC = ((nhi_e.max(axis=0) + 127) // 128).astype(np.int64)
    Cb = (nloC + nhiC).astype(np.int64)
    totch = int(Cb.sum())
    gc0 = np.concatenate([[0], np.cumsum(Cb)[:-1]]).astype(np.int64)

    # ---- per-core index/dstloc arrays ----------------------------------
    niw = totch * 8
    gidx_all, eridx_all, dstloc_all = [], [], []
    for c in range(NCOR):
        sc, dloc, blk, bloc, hi = per_core[c]
        gidx = np.zeros(totch * 128, np.int64)
        eridx = np.zeros(totch * 128, np.int64)
        dstl = np.full(totch * 128, 512.0, np.float32)
        for b in range(nblk):
            mb = blk == b
            sb_, dlb, blb, hib = sc[mb], dloc[mb], bloc[mb], hi[mb]
            lo_mask = hib == 0
            sl, dl_l, bl_l = sb_[lo_mask], dlb[lo_mask], blb[lo_mask]
            sh, dl_h, bl_h = sb_[~lo_mask], dlb[~lo_mask], blb[~lo_mask]
            if b == nblk - 1 and pad_dst:
                # fake self-edges for pad dst slots: src row 0 (lo), dst slot
                fake_b = np.arange(cfg.npc - (nblk - 1) * 128, 128)
                sl = np.concatenate([sl, np.zeros(pad_dst, np.int64)])
                dl_l = np.concatenate([dl_l, (nblk - 1) * 128 + fake_b])
                bl_l = np.concatenate([bl_l, fake_b])
            e0 = gc0[b] * 128
            nlo_pad = int(nloC[b]) * 128
            gidx[e0:e0 + len(sl)] = sl
            eridx[e0:e0 + len(sl)] = dl_l
            dstl[e0:e0 + len(sl)] = bl_l
            e1 = e0 + nlo_pad
            gidx[e1:e1 + len(sh)] = sh - cfg.losplit
            eridx[e1:e1 + len(sh)] = dl_h
            dstl[e1:e1 + len(sh)] = bl_h
            # padding slots: gidx already 0, eridx 0, dstl sentinel 512
        # wrap indices: logical i -> (i % 16, i // 16), tiled x8 partitions
        def wrap16(a):
            t = a.reshape(-1, 16).T.astype(np.int16)      # [16, totch*8]
            return np.tile(t, (8, 1))
        gidx_all.append(wrap16(gidx))
        eridx_all.append(wrap16(eridx))
        dstloc_all.append(dstl.reshape(-1, 128).T.copy())  # [128, totch]

    # ---- weights (host side prep) --------------------------------------
    nat = NAT_OF_POS
    W1r = W1.reshape(F, H, D)
    wel1 = np.einsum("khj,hj->kh", W1r, al1)
    wer1 = np.einsum("khj,hj->kh", W1r, ar1)
    W2r = W2.reshape(F, H, D)
    wel2 = np.einsum("khj,hj->kh", W2r, al2)
    wer2 = np.einsum("khj,hj->kh", W2r, ar2)

    w1 = W1[:, nat]
    rw1 = resW1[:, nat]
    w2 = W2[nat][:, nat]
    welr1 = np.concatenate([wel1, wer1], 1)                 # [256, 8]
    welr2 = np.concatenate([wel2, wer2], 1)[nat]            # rows interleaved

    kd = cfg.fd_doc // 128
    kw = cfg.fd_word_pad // 128
    wwp = np.zeros((cfg.fd_word_pad, F), np.float32)
    wwp[:cfg.fd_word] = Ww

    def ktile(w, kt):
        return np.ascontiguousarray(w.reshape(kt, 128, -1).astype(bf16))

    docn = cfg.doc_pc
    wordn = cfg.word_pc
    docn_pad = ((docn + 127) // 128) * 128
    wordn_pad = ((wordn + 127) // 128) * 128

    common = {
        "wd": ktile(Wd, kd),
        "ww": ktile(wwp, kw),
        "w1": ktile(w1, 2),
        "rw1": ktile(rw1, 2),
        "welr1": ktile(welr1, 2),
        "w2": ktile(w2, 2),
        "welr2": ktile(welr2, 2),
        "bdw": np.stack([bd.reshape(2, 128).T, bw.reshape(2, 128).T], 0).astype(np.float32),  # [2, 128, 2]
        "b1t": np.broadcast_to(b1[nat], (128, F)).astype(np.float32).copy(),
        "b2t": np.broadcast_to(b2[nat], (128, F)).astype(np.float32).copy(),
        "iota": np.broadcast_to(np.arange(128, dtype=np.float32), (128, 128)).astype(bf16).copy(),
    }

    in_maps = []
    for c in range(NCOR):
        dT = np.zeros((kd, 128, docn_pad), bf16)
        dT.reshape(cfg.fd_doc, docn_pad)[:, :docn] = (
            doc_feat[c * docn:(c + 1) * docn].T.astype(bf16))
        wT = np.zeros((kw, 128, wordn_pad), bf16)
        wT.reshape(cfg.fd_word_pad, wordn_pad)[:cfg.fd_word, :wordn] = (
            word_feat[c * wordn:(c + 1) * wordn].T.astype(bf16))
        im = dict(common)
        im.update({
            "doct": dT, "wordt": wT,
            "gidx": gidx_all[c], "eridx": eridx_all[c],
            "dstloc": dstloc_all[c],
        })
        in_maps.append(im)

    meta = dict(nloC=nloC, nhiC=nhiC, Cb=Cb, gc0=gc0, totch=totch, niw=niw,
                docn_pad=docn_pad, wordn_pad=wordn_pad, kd=kd, kw=kw)
    return in_maps, meta


# =========================================================================
# device program
# =========================================================================

def _build(cfg: Cfg, meta):
    nc = bacc.Bacc("TRN2", target_bir_lowering=False, debug=False,
                   num_devices=cfg.n_cores)

    kd, kw = meta["kd"], meta["kw"]
    docn_pad, wordn_pad = meta["docn_pad"], meta["wordn_pad"]
    nblk, npad, npc = cfg.nblk, cfg.npad, cfg.npc
    totch, niw = meta["totch"], meta["niw"]
    nloC, nhiC, Cb, gc0 = meta["nloC"], meta["nhiC"], meta["Cb"], meta["gc0"]
    cmax = int(Cb.max())
    XT = npad  # x0T padded cols

    # ---- I/O ----
    t_doct = nc.dram_tensor("doct", [kd, 128, docn_pad], BF16, kind="ExternalInput")
    t_wordt = nc.dram_tensor("wordt", [kw, 128, wordn_pad], BF16, kind="ExternalInput")
    t_wd = nc.dram_tensor("wd", [kd, 128, F], BF16, kind="ExternalInput")
    t_ww = nc.dram_tensor("ww", [kw, 128, F], BF16, kind="ExternalInput")
    t_w1 = nc.dram_tensor("w1", [2, 128, F], BF16, kind="ExternalInput")
    t_rw1 = nc.dram_tensor("rw1", [2, 128, F], BF16, kind="ExternalInput")
    t_welr1 = nc.dram_tensor("welr1", [2, 128, 8], BF16, kind="ExternalInput")
    t_w2 = nc.dram_tensor("w2", [2, 128, F], BF16, kind="ExternalInput")
    t_welr2 = nc.dram_tensor("welr2", [2, 128, 8], BF16, kind="ExternalInput")
    t_bdw = nc.dram_tensor("bdw", [2, 128, 2], F32, kind="ExternalInput")
    t_b1t = nc.dram_tensor("b1t", [128, F], F32, kind="ExternalInput")
    t_b2t = nc.dram_tensor("b2t", [128, F], F32, kind="ExternalInput")
    t_iota = nc.dram_tensor("iota", [128, 128], BF16, kind="ExternalInput")
    t_gidx = nc.dram_tensor("gidx", [128, niw], I16, kind="ExternalInput")
    t_eridx = nc.dram_tensor("eridx", [128, niw], I16, kind="ExternalInput")
    t_dstloc = nc.dram_tensor("dstloc", [128, totch], F32, kind="ExternalInput")
    t_omax = nc.dram_tensor("omax", [128, F], F32, kind="ExternalOutput")

    AF = mybir.ActivationFunctionType
    OP = mybir.AluOpType

    class _StageDone(Exception):
        pass

    with tile.TileContext(nc) as tc, ExitStack() as ctx:
      try:
        cst = ctx.enter_context(tc.tile_pool(name="cst", bufs=1))
        dram = ctx.enter_context(tc.tile_pool(name="dram", bufs=1, space="DRAM"))
        ps_big = ctx.enter_context(tc.tile_pool(name="ps_big", bufs=4, space="PSUM"))
        ps_sm = ctx.enter_context(tc.tile_pool(name="ps_sm", bufs=3, space="PSUM"))
        persist = ctx.enter_context(tc.tile_pool(name="persist", bufs=1))

        # ---- load constants ----
        def ld(pool, t, shape, dtype):
            s = pool.tile(shape, dtype, tag=f"c_{t.name}")
            src = t[:]
            if len(t.shape) == 3:
                src = src.rearrange("a p x -> p a x")
            nc.sync.dma_start(s[:], src)
            return s

        wd_sb = ld(cst, t_wd, [128, kd, F], BF16)
        ww_sb = ld(cst, t_ww, [128, kw, F], BF16)
        w1_sb = ld(cst, t_w1, [128, 2, F], BF16)
        rw1_sb = ld(cst, t_rw1, [128, 2, F], BF16)
        welr1_sb = ld(cst, t_welr1, [128, 2, 8], BF16)
        w2_sb = ld(cst, t_w2, [128, 2, F], BF16)
        welr2_sb = ld(cst, t_welr2, [128, 2, 8], BF16)
        bdw_sb = ld(cst, t_bdw, [128, 2, 2], F32)
        b1t_sb = ld(cst, t_b1t, [128, F], F32)
        b2t_sb = ld(cst, t_b2t, [128, F], F32)
        iota_sb = ld(cst, t_iota, [128, 128], BF16)
        gidx_sb = ld(cst, t_gidx, [128, niw], I16)
        eridx_sb = ld(cst, t_eridx, [128, niw], I16)
        dstloc_sb = ld(cst, t_dstloc, [128, totch], F32)

        stagep = ctx.enter_context(tc.tile_pool(name="stagep", bufs=2))

        # persistent tiles
        zpad = cst.tile([128, ERROW - 4], BF16, tag="zpad")
        nc.vector.memset(zpad[:], 0.0)

        def ertab_zero(ertab):
            for b in range(nblk):
                v = bass.AP(ertab[:].tensor, ertab[:].offset + b * 128 * ERROW + 4,
                            [[ERROW, 128], [1, ERROW - 4]])
                nc.sync.dma_start(v, zpad[:])

        x1_own = persist.tile([128, nblk, F], F32, tag="x1_own")
        er1_sb = persist.tile([128, nblk, 4], BF16, tag="er1")
        er2_sb = persist.tile([128, nblk, 4], BF16, tag="er2")
        maxacc = persist.tile([128, F], F32, tag="maxacc")

        # dram tables
        bounce1 = dram.tile([npc, ROW], mybir.dt.bfloat16, tag="bounce1")
        bounce2 = dram.tile([npc, ROW], mybir.dt.bfloat16, tag="bounce2")
        table1 = dram.tile([cfg.n, ROW], mybir.dt.bfloat16, tag="table1")
        table2 = dram.tile([cfg.n, ROW], mybir.dt.bfloat16, tag="table2")
        ertab1 = dram.tile([npad, ERROW], mybir.dt.bfloat16, tag="ertab1")
        ertab2 = dram.tile([npad, ERROW], mybir.dt.bfloat16, tag="ertab2")

        # =================== phase A: x0T ============================
        res1b = persist.tile([128, nblk, F], BF16, tag="res1b")
        xp_cm = tc.tile_pool(name="xp", bufs=1)
        xp = xp_cm.__enter__()
        x0T = xp.tile([128, 2, XT], BF16, tag="x0T")
        nc.vector.memset(x0T[:], 0.0)

        def projA(featT_sb, w_sb, kt, ncols, colbase, bias_col):
            c0 = 0
            while c0 < ncols:
                cw = min(512, ncols - c0)
                for mh in range(2):
                    psum = ps_big.tile([128, 512], F32, tag="big")
                    for k in range(kt):
                        nc.tensor.matmul(
                            psum[:, 0:cw],
                            w_sb[:, k, mh * 128:(mh + 1) * 128],
                            featT_sb[:, k, c0:c0 + cw],
                            start=(k == 0), stop=(k == kt - 1))
                    nc.scalar.activation(
                        x0T[:, mh, colbase + c0: colbase + c0 + cw],
                        psum[:, 0:cw], AF.Relu, bias=bias_col[:, mh:mh + 1])
                c0 += cw

        whalf = ((wordn_pad // 2 + 127) // 128) * 128
        with tc.tile_pool(name="featp", bufs=1) as featp:
            doct_sb = featp.tile([128, kd, docn_pad], BF16, tag="feat")
            nc.sync.dma_start(doct_sb[:], t_doct[:].rearrange("k p x -> p k x"))
            projA(doct_sb, wd_sb, kd, cfg.doc_pc, 0, bdw_sb[:, 0, :])
            for half in range(2):
                c0 = half * whalf
                cw = min(whalf, cfg.word_pc - c0)
                if cw <= 0:
                    continue
                wt = featp.tile([128, kw, whalf], BF16, tag="feat")
                nc.sync.dma_start(
                    wt[:, :, 0:cw],
                    t_wordt[:, :, c0:c0 + cw].rearrange("k p x -> p k x"))
                projA(wt, ww_sb, kw, cw, cfg.doc_pc + c0, bdw_sb[:, 1, :])

        # =================== phase D (dense per layer) ================
        def phaseD(layer, lhsT_of_block, w_sb, welr_sb, bounce, er_dst,
                   res_extra):
            """lhsT_of_block(b) -> list of two [128,128] bf16 APs."""
            for b in range(nblk):
                xt = lhsT_of_block(b)
                psum_h = ps_big.tile([128, 512], F32, tag="big")
                psum_sm = ps_sm.tile([128, 8], F32, tag="sm")
                for k in range(2):
                    nc.tensor.matmul(psum_h[:, 0:F], xt[k], w_sb[:, k, :],
                                     start=(k == 0), stop=(k == 1))
                    nc.tensor.matmul(psum_sm[:, 0:8], xt[k], welr_sb[:, k, :],
                                     start=(k == 0), stop=(k == 1))
                stage = stagep.tile([128, ROW], BF16, tag="stage")
                nc.vector.memset(stage[:, F + 4:ROW], 0.0)
                nc.scalar.copy(stage[:, 0:F], psum_h[:, 0:F])
                nc.scalar.copy(stage[:, F:F + 4], psum_sm[:, 0:4])
                nc.vector.tensor_copy(er_dst[:, b, :], psum_sm[:, 4:8])
                rows = min(128, npc - b * 128)
                nc.sync.dma_start(bounce[b * 128:b * 128 + rows, :],
                                  stage[0:rows, :])
                if res_extra is not None:
                    res_dst, rw_sb, bt_sb = res_extra
                    psum_r = ps_big.tile([128, 512], F32, tag="big")
                    for k in range(2):
                        nc.tensor.matmul(psum_r[:, 0:F], xt[k], rw_sb[:, k, :],
                                         start=(k == 0), stop=(k == 1))
                    nc.vector.tensor_tensor(out=res_dst[:, b, :], in0=psum_r[:, 0:F],
                                            in1=bt_sb[:], op=OP.add)

        phaseD(1, lambda b: [x0T[:, k, b * 128:(b + 1) * 128] for k in range(2)],
               w1_sb, welr1_sb, bounce1, er1_sb, (res1b, rw1_sb, b1t_sb))
        # er table 1 write (one strided DMA)
        ertab1_view = bass.AP(ertab1[:].tensor, ertab1[:].offset,
                              [[ERROW, 128], [128 * ERROW, nblk], [1, 4]])
        nc.sync.dma_start(ertab1_view, er1_sb[:])
        ertab_zero(ertab1)
        noag = os.environ.get("GAT_NOAG", "0") == "1"
        if noag:
            nc.sync.dma_start(table1[0:npc, :], bounce1[:])
        else:
            nc.gpsimd.collective_compute(
                "AllGather", OP.bypass,
                replica_groups=[list(range(cfg.n_cores))],
                ins=[bounce1[:]], outs=[table1[:]])
        xp_cm.__exit__(None, None, None)
        # work pools created after x0T is freed (pool footprints reserve at
        # creation in stack order)
        work = ctx.enter_context(tc.tile_pool(name="work", bufs=2))
        mkp = ctx.enter_context(tc.tile_pool(name="mkp", bufs=4))
        x1tp = ctx.enter_context(tc.tile_pool(name="x1tp", bufs=4))

        # =================== phase E (edges per layer) ================
        def phaseE(layer, table, ertab, er_own, finish_block):
            EM = int(os.environ.get("GAT_EMASK", "15"))
            for b in range(nblk):
                C = int(Cb[b]); nlo = int(nloC[b]); nhi = int(nhiC[b])
                g = work.tile([128, cmax, ROW], BF16, tag="g")
                ert = work.tile([128, cmax, ERROW], BF16, tag="ert")
                col0 = int(gc0[b]) * 8
                if EM & 1:
                    if os.environ.get("GAT_NOTAB", "0") == "1":
                        nc.vector.memset(g[:], 0.0)
                    else:
                        nc.gpsimd.dma_gather(
                            g[:, 0:nlo, :], table[0:cfg.losplit, :],
                            gidx_sb[:, col0:col0 + nlo * 8],
                            nlo * 128, nlo * 128, ROW, elem_step=ROW, single_packet=False)
                        if nhi:
                            nc.gpsimd.dma_gather(
                                g[:, nlo:C, :], table[cfg.losplit:cfg.n, :],
                                gidx_sb[:, col0 + nlo * 8: col0 + C * 8],
                                nhi * 128, nhi * 128, ROW, elem_step=ROW, single_packet=False)
                    if os.environ.get("GAT_NOER", "0") == "1":
                        nc.vector.memset(ert[:], 0.0)
                    else:
                        nc.gpsimd.dma_gather(
                            ert[:, 0:C, :], ertab[:],
                            eridx_sb[:, col0: col0 + C * 8],
                            C * 128, C * 128, ERROW, elem_step=ERROW, single_packet=False)
                else:
                    nc.vector.memset(g[:], 0.0)
                    nc.vector.memset(ert[:], 0.0)

                pt = work.tile([128, cmax * 4], BF16, tag="pt")
                if EM & 2:
                    lb = work.tile([128, cmax * 4], F32, tag="lb")
                    nc.vector.tensor_tensor(
                        out=lb[:].rearrange("p (c h) -> p c h", h=4)[:, 0:C, :],
                        in0=g[:, 0:C, F:F + 4], in1=ert[:, 0:C, 0:4], op=OP.add)
                    e1 = work.tile([128, cmax * 4], F32, tag="e1")
                    e2 = work.tile([128, cmax * 4], F32, tag="e2")
                    nc.scalar.activation(e1[:, 0:C * 4], lb[:, 0:C * 4], AF.Exp)
                    nc.scalar.activation(e2[:, 0:C * 4], lb[:, 0:C * 4], AF.Exp,
                                         scale=0.2)
                    nc.vector.tensor_tensor(out=pt[:, 0:C * 4], in0=e1[:, 0:C * 4],
                                            in1=e2[:, 0:C * 4], op=OP.max)
                else:
                    nc.vector.memset(pt[:], 1.0)
                if EM & 4:
                    ptv = pt[:].rearrange("p (c h) -> p c h", h=4)
                    pt_b = bass.AP(ptv.tensor, ptv.offset,
                                   [ptv.ap[0], [4, C], [0, D], [1, H]])
                    gv = g[:, 0:C, 0:F].rearrange("p c (j h) -> p c j h", h=H)
                    nc.vector.tensor_tensor(out=gv, in0=gv, in1=pt_b, op=OP.mult)

                tmp = work.tile([128, F], F32, tag="tmp")
                if EM & 8:
                    psf = ps_big.tile([128, 512], F32, tag="big")
                    pss = ps_sm.tile([128, 8], F32, tag="sm")
                    for c in range(C):
                        mk = mkp.tile([128, 128], BF16, tag="mk")
                        nc.vector.tensor_scalar(
                            mk[:], iota_sb[:],
                            dstloc_sb[:, gc0[b] + c: gc0[b] + c + 1], None,
                            OP.is_equal)
                        nc.tensor.matmul(psf[:, 0:F], mk[:], g[:, c, 0:F],
                                         start=(c == 0), stop=(c == C - 1))
                        nc.tensor.matmul(pss[:, 0:4], mk[:], pt[:, c * 4:(c + 1) * 4],
                                         start=(c == 0), stop=(c == C - 1))
                    r = work.tile([128, 4], F32, tag="r")
                    nc.vector.reciprocal(r[:], pss[:, 0:4])
                    r_b = bass.AP(r[:].tensor, r[:].offset,
                                  [r[:].ap[0], [0, D], [1, H]])
                    nc.vector.tensor_tensor(
                        out=tmp[:].rearrange("p (j h) -> p j h", h=H),
                        in0=psf[:, 0:F].rearrange("p (j h) -> p j h", h=H),
                        in1=r_b, op=OP.mult)
                else:
                    nc.vector.tensor_copy(tmp[:], g[:, 0, 0:F])
                finish_block(b, tmp)

        # ---- layer 1 finish: x1 = relu(tmp + res1b) ----
        def fin1(b, tmp):
            nc.vector.tensor_tensor(out=tmp[:], in0=tmp[:],
                                    in1=res1b[:, b, :], op=OP.add)
            nc.scalar.activation(x1_own[:, b, :], tmp[:], AF.Relu)

        stage_lim = int(os.environ.get("GAT_STAGE", "4"))
        if stage_lim >= 2:
            phaseE(1, table1, ertab1, er1_sb, fin1)

        # =================== phase D2 + AllGather 2 ===================
        if stage_lim < 2:
            nc.vector.memset(x1_own[:], 0.0)
        ident = cst.tile([128, 128], F32)
        from concourse.masks import make_identity
        make_identity(nc, ident[:])

        def lhsT2(b):
            outs = []
            for k in range(2):
                pst = ps_big.tile([128, 512], F32, tag="big")
                nc.tensor.transpose(pst[:, 0:128], x1_own[:, b, k * 128:(k + 1) * 128],
                                    ident[:])
                xt = x1tp.tile([128, 128], BF16, tag="x1t")
                nc.scalar.copy(xt[:], pst[:, 0:128])
                outs.append(xt[:])
            return outs

        if stage_lim < 3:
            nc.vector.memset(maxacc[:], 0.0)
            nc.vector.tensor_tensor(out=maxacc[:], in0=x1_own[:, 0, :],
                                    in1=res1b[:, 0, :], op=OP.add)
            nc.sync.dma_start(t_omax[:], maxacc[:])
            raise _StageDone()
        phaseD(2, lhsT2, w2_sb, welr2_sb, bounce2, er2_sb, None)
        ertab2_view = bass.AP(ertab2[:].tensor, ertab2[:].offset,
                              [[ERROW, 128], [128 * ERROW, nblk], [1, 4]])
        nc.sync.dma_start(ertab2_view, er2_sb[:])
        ertab_zero(ertab2)
        if noag:
            nc.sync.dma_start(table2[0:npc, :], bounce2[:])
        else:
            nc.gpsimd.collective_compute(
                "AllGather", OP.bypass,
                replica_groups=[list(range(cfg.n_cores))],
                ins=[bounce2[:]], outs=[table2[:]])
        # res2b = x1_own + b2t (in place, after table2 inputs are produced)
        for b in range(nblk):
            nc.vector.tensor_tensor(out=x1_own[:, b, :], in0=x1_own[:, b, :],
                                    in1=b2t_sb[:], op=OP.add)

        # =================== phase E2 + max ===========================
        nc.vector.memset(maxacc[:], -1e30)
        lastp = npc - (nblk - 1) * 128   # real rows in last block

        def fin2(b, tmp):
            nc.vector.tensor_tensor(out=tmp[:], in0=tmp[:],
                                    in1=x1_own[:, b, :], op=OP.add)
            x2 = work.tile([128, F], F32, tag="x2")
            nc.scalar.activation(x2[:], tmp[:], AF.Relu)
            np_ = 128 if b < nblk - 1 else lastp
            nc.vector.tensor_tensor(out=maxacc[0:np_, :], in0=maxacc[0:np_, :],
                                    in1=x2[0:np_, :], op=OP.max)

        if stage_lim >= 4:
            phaseE(2, table2, ertab2, er2_sb, fin2)
        nc.sync.dma_start(t_omax[:], maxacc[:])
      except _StageDone:
        pass

    nc.compile()
    return nc


# =========================================================================
# entry point
# =========================================================================

_REAL_CFG = Cfg()


def _run(cfg: Cfg, inputs, trace=False):
    in_maps, meta = _host_prep(cfg, inputs)
    nc = _build(cfg, meta)
    res = run_bass_kernel_spmd(
        nc, in_maps, core_ids=list(range(cfg.n_cores)),
        trace=trace)
    omax = np.stack([r["omax"] for r in res.results])      # [cores, 128, F]
    pooled_ilv = omax.max(axis=(0, 1))
    pooled = pooled_ilv[POS_OF_NAT]                        # back to natural
    return pooled.astype(np.float32), res


def kernel(**inputs):
    cfg = _REAL_CFG
    pooled, _ = _run(cfg, inputs, trace=bool(int(os.environ.get("GAT_TRACE", "0"))))
    Ws = _f32(inputs["Ws"]); bs = _f32(inputs["bs"])
    Wc = _f32(inputs["Wc"]); bc = _f32(inputs["bc"])
    h = np.maximum(pooled @ Ws + bs, 0.0)
    out = h @ Wc + bc
    return out.reshape(1, 2).astype(np.float32)



# revision 6
# speedup vs baseline: 2.0386x; 2.0386x over previous
"""Trainium2 Bass kernel for a 2-layer GAT (CeleTrip word-doc graph).

Contract: kernel(**inputs) takes the full unsharded inputs of
reference.setup_inputs() and returns the full [1, 2] float32 output.

Strategy (8 NeuronCores), v2:
 - Node permutation pi balances doc/word rows per core; core c owns pi-nodes
   [c*NPC, (c+1)*NPC) as edge-softmax destinations.
 - Dense projections sharded by node; per layer the per-node table
   [h(256 interleaved) | el(4) | pad] (bf16, 768B rows) is AllGather'd
   (Shared output) so every core can gather arbitrary source rows with
   dma_gather (int16 indices -> table split lo/hi at row 32768).
 - Edge phase: edges (self-loops excluded) sorted by dst block; groups of
   G=3 blocks share one lo + one hi dma_gather call.  Per 128-edge chunk a
   HOST-PRECOMPUTED one-hot mask (fp8) and its transpose are streamed from
   DRAM.  er per edge comes from a tiny PE matmul maskT^T @ er_block; then
   p = exp(leaky_relu(el+er)) = max(exp(L), exp(0.2 L)) overwrites the el
   columns of the gathered rows, features are premultiplied by p, and ONE
   matmul per chunk accumulates [p*h | p] into PSUM (psf/pss fused).
 - Self-loop contribution added per block from the core's own bounce rows
   (sequential readback), avoiding 50k gather descriptors.
 - Final graph max-pool: per-core partition-wise running max -> host max +
   tiny host MLP.
"""

import math
import os
from contextlib import ExitStack
from dataclasses import dataclass

import numpy as np
import ml_dtypes

import concourse.bass as bass
import concourse.mybir as mybir
import concourse.tile as tile
import concourse.bacc as bacc
from concourse.bass_utils import run_bass_kernel_spmd

bf16 = ml_dtypes.bfloat16
f8 = ml_dtypes.float8_e4m3fn
F32 = mybir.dt.float32
BF16 = mybir.dt.bfloat16
FP8 = mybir.dt.float8e4
I16 = mybir.dt.int16

H = 4
D = 64
F = H * D            # 256
FE = F + 4           # h + el (later h + p)
ROW = 384            # table row (bf16 elems): 256 h + 4 el + pad -> 768B
G = 3                # dst blocks per gather group


@dataclass
class Cfg:
    n_doc: int = 5000
    n_word: int = 45000
    n_cores: int = 8
    fd_doc: int = 768
    fd_word: int = 300
    losplit: int = 32768

    @property
    def n(self):
        return self.n_doc + self.n_word

    @property
    def npc(self):
        return self.n // self.n_cores

    @property
    def doc_pc(self):
        return self.n_doc // self.n_cores

    @property
    def word_pc(self):
        return self.n_word // self.n_cores

    @property
    def nblk(self):
        return (self.npc + 127) // 128

    @property
    def npad(self):
        return self.nblk * 128

    @property
    def fd_word_pad(self):
        return ((self.fd_word + 127) // 128) * 128


# interleave permutation: position p in interleaved layout holds natural
# feature (p % 4) * 64 + p // 4   (i.e. [j, h] order instead of [h, j])
NAT_OF_POS = (np.arange(F) % H) * D + np.arange(F) // H     # ilv <- nat
POS_OF_NAT = np.argsort(NAT_OF_POS)                          # nat -> ilv


def _f32(x):
    return np.ascontiguousarray(np.asarray(x), dtype=np.float32)


def _perm_old2new(cfg: Cfg):
    """old node id -> pi node id."""
    p = np.empty(cfg.n, np.int64)
    d = np.arange(cfg.n_doc)
    p[d] = (d // cfg.doc_pc) * cfg.npc + d % cfg.doc_pc
    w = np.arange(cfg.n_word)
    p[cfg.n_doc + w] = (w // cfg.word_pc) * cfg.npc + cfg.doc_pc + w % cfg.word_pc
    return p


def _edge_structs(cfg: Cfg, src, dst):
    """Group structure + per-core gidx/mask arrays (graph-static)."""
    perm = _perm_old2new(cfg)
    nonself = src != dst
    s = perm[src[nonself]]
    d = perm[dst[nonself]]

    owner = d // cfg.npc
    NCOR = cfg.n_cores
    nblk = cfg.nblk
    per_core = []
    for c in range(NCOR):
        m = owner == c
        sc = s[m]
        dloc = d[m] - c * cfg.npc
        blk = dloc // 128
        hi = (sc >= cfg.losplit).astype(np.int64)
        per_core.append((sc, dloc, blk, hi))

    nlo_e = np.zeros((NCOR, nblk), np.int64)
    nhi_e = np.zeros((NCOR, nblk), np.int64)
    for c in range(NCOR):
        sc, dloc, blk, hi = per_core[c]
        hicnt = np.bincount(blk[hi == 1], minlength=nblk)
        allcnt = np.bincount(blk, minlength=nblk)
        nhi_e[c] = hicnt
        nlo_e[c] = allcnt - hicnt
    nloC = np.maximum(1, (nlo_e.max(axis=0) + 127) // 128).astype(np.int64)
    nhiC = ((nhi_e.max(axis=0) + 127) // 128).astype(np.int64)

    ngrp = (nblk + G - 1) // G
    grp_blocks = [list(range(g * G, min((g + 1) * G, nblk))) for g in range(ngrp)]
    ch_of = {}
    flat = 0
    grp_c0, grp_lo, grp_hi = [], [], []
    for g, blocks in enumerate(grp_blocks):
        grp_c0.append(flat)
        for b in blocks:
            ch_of[(b, 0)] = flat
            flat += int(nloC[b])
        grp_lo.append(flat - grp_c0[g])
        for b in blocks:
            ch_of[(b, 1)] = flat
            flat += int(nhiC[b])
        grp_hi.append(flat - grp_c0[g] - grp_lo[g])
    totch = flat
    grp_c = [grp_lo[g] + grp_hi[g] for g in range(ngrp)]
    cmaxg = max(grp_c)

    gidx_all, mask_all, maskT_all = [], [], []
    base_lo = np.array([ch_of[(b, 0)] for b in range(nblk)], np.int64)
    base_hi = np.array([ch_of[(b, 1)] for b in range(nblk)], np.int64)
    for c in range(NCOR):
        sc, dloc, blk, hi = per_core[c]
        gidx = np.zeros(totch * 128, np.int64)
        msk = np.zeros((totch, 128, 128), np.uint8)
        for hv, base in ((0, base_lo), (1, base_hi)):
            mh = hi == hv
            sb_, dlb, blb = sc[mh], dloc[mh], blk[mh]
            order = np.argsort(blb, kind='stable')
            sb_, dlb, blb = sb_[order], dlb[order], blb[order]
            cnt = np.bincount(blb, minlength=nblk)
            off = np.concatenate([[0], np.cumsum(cnt)[:-1]])
            rank = np.arange(len(blb)) - off[blb]
            chunk = base[blb] + rank // 128
            pos = rank % 128
            gidx[chunk * 128 + pos] = sb_ - (cfg.losplit if hv else 0)
            msk[chunk, pos, dlb % 128] = 1
        t = gidx.reshape(-1, 16).T.astype(np.int16)
        gidx_all.append(np.tile(t, (8, 1)))
        mask_all.append(np.ascontiguousarray(
            msk.transpose(1, 0, 2)).astype(f8))          # [128pos, totch, 128slot]
        maskT_all.append(np.ascontiguousarray(
            msk.transpose(2, 0, 1)).astype(f8))          # [128slot, totch, 128pos]

    meta = dict(nloC=nloC, nhiC=nhiC, totch=totch, niw=totch * 8,
                ngrp=ngrp, grp_blocks=grp_blocks, grp_c0=grp_c0,
                grp_lo=grp_lo, grp_hi=grp_hi, grp_c=grp_c, cmaxg=cmaxg,
                ch_of=ch_of)
    return gidx_all, mask_all, maskT_all, meta


def _host_prep(cfg: Cfg, inputs):
    doc_feat = _f32(inputs["doc_feat"])
    word_feat = _f32(inputs["word_feat"])
    src = np.asarray(inputs["src"]).astype(np.int64)
    dst = np.asarray(inputs["dst"]).astype(np.int64)

    Wd = _f32(inputs["Wd"]); bd = _f32(inputs["bd"])
    Ww = _f32(inputs["Ww"]); bw = _f32(inputs["bw"])
    W1 = _f32(inputs["W1"]); al1 = _f32(inputs["al1"]); ar1 = _f32(inputs["ar1"])
    b1 = _f32(inputs["b1"]); resW1 = _f32(inputs["resW1"])
    W2 = _f32(inputs["W2"]); al2 = _f32(inputs["al2"]); ar2 = _f32(inputs["ar2"])
    b2 = _f32(inputs["b2"])

    gidx_all, mask_all, maskT_all, meta = _edge_structs(cfg, src, dst)

    nat = NAT_OF_POS
    W1r = W1.reshape(F, H, D)
    wel1 = np.einsum("khj,hj->kh", W1r, al1)
    wer1 = np.einsum("khj,hj->kh", W1r, ar1)
    W2r = W2.reshape(F, H, D)
    wel2 = np.einsum("khj,hj->kh", W2r, al2)
    wer2 = np.einsum("khj,hj->kh", W2r, ar2)

    w1c = np.concatenate([W1[:, nat], wel1, wer1], 1)          # [256, 264]
    w2c = np.concatenate([W2[:, nat], wel2, wer2], 1)[nat]
    rw1 = resW1[:, nat]

    kd = cfg.fd_doc // 128
    kw = cfg.fd_word_pad // 128
    wwp = np.zeros((cfg.fd_word_pad, F), np.float32)
    wwp[:cfg.fd_word] = Ww

    def ktile(w, kt):
        return np.ascontiguousarray(w.reshape(kt, 128, -1).astype(bf16))

    docn = cfg.doc_pc
    wordn = cfg.word_pc
    docn_pad = ((docn + 127) // 128) * 128
    wordn_pad = ((wordn + 127) // 128) * 128

    common = {
        "wd": ktile(Wd, kd),
        "ww": ktile(wwp, kw),
        "w1c": ktile(w1c, 2),
        "rw1": ktile(rw1, 2),
        "w2c": ktile(w2c, 2),
        "bdw": np.stack([bd.reshape(2, 128).T, bw.reshape(2, 128).T], 0).astype(np.float32),
        "b1t": np.broadcast_to(b1[nat], (128, F)).astype(np.float32).copy(),
        "b2t": np.broadcast_to(b2[nat], (128, F)).astype(np.float32).copy(),
    }

    in_maps = []
    for c in range(cfg.n_cores):
        dT = np.zeros((kd, 128, docn_pad), bf16)
        dT.reshape(cfg.fd_doc, docn_pad)[:, :docn] = (
            doc_feat[c * docn:(c + 1) * docn].T.astype(bf16))
        wT = np.zeros((kw, 128, wordn_pad), bf16)
        wT.reshape(cfg.fd_word_pad, wordn_pad)[:cfg.fd_word, :wordn] = (
            word_feat[c * wordn:(c + 1) * wordn].T.astype(bf16))
        im = dict(common)
        im.update({
            "doct": dT, "wordt": wT,
            "gidx": gidx_all[c],
            "mask": mask_all[c], "maskT": maskT_all[c],
        })
        in_maps.append(im)

    meta.update(docn_pad=docn_pad, wordn_pad=wordn_pad, kd=kd, kw=kw)
    return in_maps, meta


# =========================================================================
# device program
# =========================================================================

def _build(cfg: Cfg, meta):
    nc = bacc.Bacc("TRN2", target_bir_lowering=False, debug=False,
                   num_devices=cfg.n_cores)

    kd, kw = meta["kd"], meta["kw"]
    docn_pad, wordn_pad = meta["docn_pad"], meta["wordn_pad"]
    nblk, npad, npc = cfg.nblk, cfg.npad, cfg.npc
    totch, niw = meta["totch"], meta["niw"]
    nloC, nhiC = meta["nloC"], meta["nhiC"]
    ngrp, grp_blocks = meta["ngrp"], meta["grp_blocks"]
    grp_c0, grp_lo, grp_hi, grp_c = (meta["grp_c0"], meta["grp_lo"],
                                     meta["grp_hi"], meta["grp_c"])
    cmaxg = meta["cmaxg"]
    ch_of = meta["ch_of"]
    XT = npad

    t_doct = nc.dram_tensor("doct", [kd, 128, docn_pad], BF16, kind="ExternalInput")
    t_wordt = nc.dram_tensor("wordt", [kw, 128, wordn_pad], BF16, kind="ExternalInput")
    t_wd = nc.dram_tensor("wd", [kd, 128, F], BF16, kind="ExternalInput")
    t_ww = nc.dram_tensor("ww", [kw, 128, F], BF16, kind="ExternalInput")
    t_w1c = nc.dram_tensor("w1c", [2, 128, F + 8], BF16, kind="ExternalInput")
    t_rw1 = nc.dram_tensor("rw1", [2, 128, F], BF16, kind="ExternalInput")
    t_w2c = nc.dram_tensor("w2c", [2, 128, F + 8], BF16, kind="ExternalInput")
    t_bdw = nc.dram_tensor("bdw", [2, 128, 2], F32, kind="ExternalInput")
    t_b1t = nc.dram_tensor("b1t", [128, F], F32, kind="ExternalInput")
    t_b2t = nc.dram_tensor("b2t", [128, F], F32, kind="ExternalInput")
    t_gidx = nc.dram_tensor("gidx", [128, niw], I16, kind="ExternalInput")
    t_mask = nc.dram_tensor("mask", [128, totch, 128], FP8, kind="ExternalInput")
    t_maskT = nc.dram_tensor("maskT", [128, totch, 128], FP8, kind="ExternalInput")
    t_omax = nc.dram_tensor("omax", [128, F], F32, kind="ExternalOutput")

    AF = mybir.ActivationFunctionType
    OP = mybir.AluOpType

    class _StageDone(Exception):
        pass

    with tile.TileContext(nc) as tc, ExitStack() as ctx:
      try:
        cst = ctx.enter_context(tc.tile_pool(name="cst", bufs=1))
        dram = ctx.enter_context(tc.tile_pool(name="dram", bufs=1, space="DRAM"))
        ps_d = ctx.enter_context(tc.tile_pool(name="ps_d", bufs=2, space="PSUM"))
        ps_f = ctx.enter_context(tc.tile_pool(name="ps_f", bufs=2, space="PSUM"))
        ps_er = ctx.enter_context(tc.tile_pool(name="ps_er", bufs=2, space="PSUM"))
        persist = ctx.enter_context(tc.tile_pool(name="persist", bufs=1))

        def ld(pool, t, shape, dtype):
            s = pool.tile(shape, dtype, tag=f"c_{t.name}")
            srcap = t[:]
            if len(t.shape) == 3:
                srcap = srcap.rearrange("a p x -> p a x")
            nc.sync.dma_start(s[:], srcap)
            return s

        wd_sb = ld(cst, t_wd, [128, kd, F], BF16)
        ww_sb = ld(cst, t_ww, [128, kw, F], BF16)
        w1c_sb = ld(cst, t_w1c, [128, 2, F + 8], BF16)
        rw1_sb = ld(cst, t_rw1, [128, 2, F], BF16)
        w2c_sb = ld(cst, t_w2c, [128, 2, F + 8], BF16)
        bdw_sb = ld(cst, t_bdw, [128, 2, 2], F32)
        b1t_sb = ld(cst, t_b1t, [128, F], F32)
        b2t_sb = ld(cst, t_b2t, [128, F], F32)
        gidx_sb = ld(cst, t_gidx, [128, niw], I16)

        stagep = ctx.enter_context(tc.tile_pool(name="stagep", bufs=2))

        x1_own = persist.tile([128, nblk, F], BF16, tag="x1_own")
        er1_sb = persist.tile([128, nblk, 4], BF16, tag="er1")
        er2_sb = persist.tile([128, nblk, 4], BF16, tag="er2")
        maxacc = persist.tile([128, F], F32, tag="maxacc")

        bounce1 = dram.tile([npad, ROW], BF16, tag="bounce1")
        bounce2 = dram.tile([npad, ROW], BF16, tag="bounce2")
        resb = dram.tile([nblk, 128, F], BF16, tag="resb")
        table1 = nc.dram_tensor("table1", [cfg.n, ROW], BF16,
                                kind="Internal", addr_space="Shared")
        table2 = nc.dram_tensor("table2", [cfg.n, ROW], BF16,
                                kind="Internal", addr_space="Shared")

        # zero the bounce pad rows once (own-row readback reads them)
        zpad = cst.tile([128, ROW], BF16, tag="zpad")
        nc.vector.memset(zpad[:], 0.0)
        padrows = npad - npc
        if padrows:
            nc.sync.dma_start(bounce1[npc:npad, :], zpad[0:padrows, :])
            nc.sync.dma_start(bounce2[npc:npad, :], zpad[0:padrows, :])

        # =================== phase A: x0T ============================
        xp_cm = tc.tile_pool(name="xp", bufs=1)
        xp = xp_cm.__enter__()
        x0T = xp.tile([128, 2, XT], BF16, tag="x0T")
        nc.vector.memset(x0T[:], 0.0)

        def projA(featT_sb, w_sb, kt, ncols, colbase, bias_col):
            c0 = 0
            while c0 < ncols:
                cw = min(512, ncols - c0)
                for mh in range(2):
                    psum = ps_d.tile([128, 512], F32, tag="d")
                    for k in range(kt):
                        nc.tensor.matmul(
                            psum[:, 0:cw],
                            w_sb[:, k, mh * 128:(mh + 1) * 128],
                            featT_sb[:, k, c0:c0 + cw],
                            start=(k == 0), stop=(k == kt - 1))
                    nc.scalar.activation(
                        x0T[:, mh, colbase + c0: colbase + c0 + cw],
                        psum[:, 0:cw], AF.Relu, bias=bias_col[:, mh:mh + 1])
                c0 += cw

        whalf = ((wordn_pad // 2 + 127) // 128) * 128
        with tc.tile_pool(name="featp", bufs=1) as featp:
            doct_sb = featp.tile([128, kd, docn_pad], BF16, tag="feat")
            nc.sync.dma_start(doct_sb[:], t_doct[:].rearrange("k p x -> p k x"))
            projA(doct_sb, wd_sb, kd, cfg.doc_pc, 0, bdw_sb[:, 0, :])
            for half in range(2):
                c0 = half * whalf
                cw = min(whalf, cfg.word_pc - c0)
                if cw <= 0:
                    continue
                wt = featp.tile([128, kw, whalf], BF16, tag="feat")
                nc.sync.dma_start(
                    wt[:, :, 0:cw],
                    t_wordt[:, :, c0:c0 + cw].rearrange("k p x -> p k x"))
                projA(wt, ww_sb, kw, cw, cfg.doc_pc + c0, bdw_sb[:, 1, :])

        # =================== phase D (dense per layer) ================
        def phaseD(layer, lhsT_of_block, wc_sb, bounce, er_dst, res_extra):
            for b in range(nblk):
                xt = lhsT_of_block(b)
                psum_h = ps_d.tile([128, 512], F32, tag="d")
                for k in range(2):
                    nc.tensor.matmul(psum_h[:, 0:F + 8], xt[k], wc_sb[:, k, :],
                                     start=(k == 0), stop=(k == 1))
                stage = stagep.tile([128, ROW], BF16, tag="stage")
                nc.scalar.copy(stage[:, 0:FE], psum_h[:, 0:FE])
                nc.vector.tensor_copy(er_dst[:, b, :], psum_h[:, F + 4:F + 8])
                rows = min(128, npc - b * 128)
                nc.sync.dma_start(bounce[b * 128:b * 128 + rows, :],
                                  stage[0:rows, :])
                if res_extra is not None:
                    rw_sb, bt_sb = res_extra
                    psum_r = ps_d.tile([128, 512], F32, tag="d")
                    for k in range(2):
                        nc.tensor.matmul(psum_r[:, 0:F], xt[k], rw_sb[:, k, :],
                                         start=(k == 0), stop=(k == 1))
                    rst = stagep.tile([128, F], BF16, tag="rst")
                    nc.vector.tensor_tensor(out=rst[:], in0=psum_r[:, 0:F],
                                            in1=bt_sb[:], op=OP.add)
                    nc.sync.dma_start(resb[b], rst[:])

        phaseD(1, lambda b: [x0T[:, k, b * 128:(b + 1) * 128] for k in range(2)],
               w1c_sb, bounce1, er1_sb, (rw1_sb, b1t_sb))
        nc.gpsimd.collective_compute(
            "AllGather", OP.bypass,
            replica_groups=[list(range(cfg.n_cores))],
            ins=[bounce1[0:npc, :]], outs=[table1[:]])
        xp_cm.__exit__(None, None, None)

        work = ctx.enter_context(tc.tile_pool(name="work", bufs=2))
        mkp = ctx.enter_context(tc.tile_pool(name="mkp", bufs=2))
        ownp = ctx.enter_context(tc.tile_pool(name="ownp", bufs=2))
        smp = ctx.enter_context(tc.tile_pool(name="smp", bufs=3))
        numt = ctx.enter_context(tc.tile_pool(name="numt", bufs=2 * G))
        x1tp = ctx.enter_context(tc.tile_pool(name="x1tp", bufs=4))

        # =================== phase E (edges per layer) ================
        def phaseE(layer, table, er_sb, bounce, finish_block):
            for g in range(ngrp):
                blocks = grp_blocks[g]
                nb = len(blocks)
                c0 = grp_c0[g]
                nlo_g, nhi_g, nc_g = grp_lo[g], grp_hi[g], grp_c[g]
                gt = work.tile([128, cmaxg, ROW], BF16, tag="g")
                nc.gpsimd.dma_gather(
                    gt[:, 0:nlo_g, :], table[0:cfg.losplit, :],
                    gidx_sb[:, c0 * 8: (c0 + nlo_g) * 8],
                    nlo_g * 128, nlo_g * 128, ROW, elem_step=ROW,
                    single_packet=False)
                if nhi_g:
                    nc.gpsimd.dma_gather(
                        gt[:, nlo_g:nc_g, :], table[cfg.losplit:cfg.n, :],
                        gidx_sb[:, (c0 + nlo_g) * 8: (c0 + nc_g) * 8],
                        nhi_g * 128, nhi_g * 128, ROW, elem_step=ROW,
                        single_packet=False)
                mk = mkp.tile([128, cmaxg, 128], FP8, tag="mk")
                nc.sync.dma_start(mk[:, 0:nc_g, :], t_mask[:, c0:c0 + nc_g, :])
                mkT = mkp.tile([128, cmaxg, 128], FP8, tag="mkT")
                nc.sync.dma_start(mkT[:, 0:nc_g, :], t_maskT[:, c0:c0 + nc_g, :])

                # own rows + residual rows for this group's blocks
                own = ownp.tile([128, G, FE], BF16, tag="own")
                r0 = blocks[0] * 128
                nc.sync.dma_start(
                    own[:, 0:nb, :],
                    bounce[r0:r0 + nb * 128, 0:FE]
                    .rearrange("(b p) f -> p b f", p=128))
                if layer == 1:
                    resg = ownp.tile([128, G, F], BF16, tag="resg")
                    nc.sync.dma_start(
                        resg[:, 0:nb, :],
                        resb[blocks[0]:blocks[0] + nb]
                        .rearrange("b p f -> p b f"))
                else:
                    resg = None

                # pself: [128, nb, 4]
                pself = ownp.tile([128, G, 4], F32, tag="pself")
                lbs = smp.tile([128, G * 4], F32, tag="lbs")
                nc.vector.tensor_tensor(
                    out=lbs[:].rearrange("p (b h) -> p b h", h=4)[:, 0:nb, :],
                    in0=own[:, 0:nb, F:FE],
                    in1=er_sb[:, blocks[0]:blocks[0] + nb, :],
                    op=OP.add)
                es1 = smp.tile([128, G * 4], F32, tag="es1")
                es2 = smp.tile([128, G * 4], F32, tag="es2")
                nc.scalar.activation(es1[:, 0:nb * 4], lbs[:, 0:nb * 4], AF.Exp)
                nc.scalar.activation(es2[:, 0:nb * 4], lbs[:, 0:nb * 4], AF.Exp,
                                     scale=0.2)
                nc.vector.tensor_tensor(
                    out=pself[:, 0:nb, :],
                    in0=es1[:].rearrange("p (b h) -> p b h", h=4)[:, 0:nb, :],
                    in1=es2[:].rearrange("p (b h) -> p b h", h=4)[:, 0:nb, :],
                    op=OP.max)

                den_g = smp.tile([128, G * 4], F32, tag="den_g")
                num2s = []
                for bi, b in enumerate(blocks):
                    spans = [(ch_of[(b, 0)] - c0, int(nloC[b]))]
                    if nhiC[b]:
                        spans.append((ch_of[(b, 1)] - c0, int(nhiC[b])))
                    for (s0, ns) in spans:
                        pse = ps_er.tile([128, 128], F32, tag="er")
                        for j in range(ns):
                            nc.tensor.matmul(
                                pse[:, j * 4:(j + 1) * 4],
                                mkT[:, s0 + j, :], er_sb[:, b, :],
                                start=True, stop=True)
                        lb = smp.tile([128, 128], F32, tag="lb")
                        gel = gt[:, s0:s0 + ns, F:FE]
                        nc.vector.tensor_tensor(
                            out=lb[:].rearrange("p (c h) -> p c h", h=4)[:, 0:ns, :],
                            in0=gel,
                            in1=pse[:].rearrange("p (c h) -> p c h", h=4)[:, 0:ns, :],
                            op=OP.add)
                        e1 = smp.tile([128, 128], F32, tag="e1")
                        e2 = smp.tile([128, 128], F32, tag="e2")
                        nc.scalar.activation(e1[:, 0:ns * 4], lb[:, 0:ns * 4], AF.Exp)
                        nc.scalar.activation(e2[:, 0:ns * 4], lb[:, 0:ns * 4], AF.Exp,
                                             scale=0.2)
                        nc.vector.tensor_tensor(
                            out=gel,
                            in0=e1[:].rearrange("p (c h) -> p c h", h=4)[:, 0:ns, :],
                            in1=e2[:].rearrange("p (c h) -> p c h", h=4)[:, 0:ns, :],
                            op=OP.max)
                        gv = gt[:, s0:s0 + ns, 0:F].rearrange(
                            "p c (j h) -> p c j h", h=H)
                        pv = gt[:, s0:s0 + ns, F:FE]
                        p_b = bass.AP(pv.tensor, pv.offset,
                                      [pv.ap[0], pv.ap[1], [0, D], [1, H]])
                        nc.vector.tensor_tensor(out=gv, in0=gv, in1=p_b, op=OP.mult)

                    psf = ps_f.tile([128, FE], F32, tag="f")
                    allch = [s for (s0, ns) in spans for s in range(s0, s0 + ns)]
                    for ci, ch in enumerate(allch):
                        nc.tensor.matmul(psf[:, 0:FE], mk[:, ch, :],
                                         gt[:, ch, 0:FE],
                                         start=(ci == 0), stop=(ci == len(allch) - 1))

                    num2 = numt.tile([128, F], F32, tag="num2")
                    t0v = own[:, bi, 0:F].rearrange("p (j h) -> p j h", h=H)
                    psv = pself[:, bi, :]
                    ps_b = bass.AP(psv.tensor, psv.offset,
                                   [psv.ap[0], [0, D], [1, H]])
                    nc.vector.tensor_tensor(
                        out=num2[:].rearrange("p (j h) -> p j h", h=H),
                        in0=t0v, in1=ps_b, op=OP.mult)
                    nc.vector.tensor_tensor(out=num2[:], in0=num2[:],
                                            in1=psf[:, 0:F], op=OP.add)
                    nc.vector.tensor_tensor(out=den_g[:, bi * 4:(bi + 1) * 4],
                                            in0=psf[:, F:FE],
                                            in1=pself[:, bi, :], op=OP.add)
                    num2s.append(num2)
                r_g = smp.tile([128, G * 4], F32, tag="r_g")
                nc.vector.reciprocal(r_g[:, 0:nb * 4], den_g[:, 0:nb * 4])
                for bi, b in enumerate(blocks):
                    rv = r_g[:, bi * 4:(bi + 1) * 4]
                    r_b = bass.AP(rv.tensor, rv.offset,
                                  [rv.ap[0], [0, D], [1, H]])
                    num2 = num2s[bi]
                    tmp = smp.tile([128, F], F32, tag="tmp")
                    nc.vector.tensor_tensor(
                        out=tmp[:].rearrange("p (j h) -> p j h", h=H),
                        in0=num2[:].rearrange("p (j h) -> p j h", h=H),
                        in1=r_b, op=OP.mult)
                    finish_block(b, bi, tmp, resg)

        # ---- layer 1 finish: x1 = relu(tmp + res1) ----
        def fin1(b, bi, tmp, resg):
            nc.vector.tensor_tensor(out=tmp[:], in0=tmp[:],
                                    in1=resg[:, bi, :], op=OP.add)
            nc.vector.tensor_scalar(x1_own[:, b, :], tmp[:], 0.0, None, OP.max)

        stage_lim = int(os.environ.get("GAT_STAGE", "4"))
        if stage_lim >= 2:
            phaseE(1, table1, er1_sb, bounce1, fin1)

        # =================== phase D2 + AllGather 2 ===================
        if stage_lim < 2:
            nc.vector.memset(x1_own[:], 0.0)
        ident = cst.tile([128, 128], F32)
        from concourse.masks import make_identity
        make_identity(nc, ident[:])

        def lhsT2(b):
            x1f = x1tp.tile([128, F], F32, tag="x1f")
            nc.scalar.copy(x1f[:], x1_own[:, b, :])
            outs = []
            for k in range(2):
                pst = ps_d.tile([128, 512], F32, tag="d")
                nc.tensor.transpose(pst[:, 0:128], x1f[:, k * 128:(k + 1) * 128],
                                    ident[:])
                xt = x1tp.tile([128, 128], BF16, tag="x1t")
                nc.scalar.copy(xt[:], pst[:, 0:128])
                outs.append(xt[:])
            return outs

        if stage_lim < 3:
            nc.vector.memset(maxacc[:], 0.0)
            nc.vector.tensor_tensor(out=maxacc[:], in0=x1_own[:, 0, :],
                                    in1=b1t_sb[:], op=OP.add)
            nc.sync.dma_start(t_omax[:], maxacc[:])
            raise _StageDone()
        phaseD(2, lhsT2, w2c_sb, bounce2, er2_sb, None)
        nc.gpsimd.collective_compute(
            "AllGather", OP.bypass,
            replica_groups=[list(range(cfg.n_cores))],
            ins=[bounce2[0:npc, :]], outs=[table2[:]])
        # x1b = x1_own + b2t (in place, after table2 inputs are produced)
        for b in range(nblk):
            nc.vector.tensor_tensor(out=x1_own[:, b, :], in0=x1_own[:, b, :],
                                    in1=b2t_sb[:], op=OP.add)

        # =================== phase E2 + max ===========================
        nc.vector.memset(maxacc[:], -1e30)
        lastp = npc - (nblk - 1) * 128

        def fin2(b, bi, tmp, resg):
            nc.vector.tensor_tensor(out=tmp[:], in0=tmp[:],
                                    in1=x1_own[:, b, :], op=OP.add)
            x2 = smp.tile([128, F], F32, tag="x2")
            nc.vector.tensor_scalar(x2[:], tmp[:], 0.0, None, OP.max)
            np_ = 128 if b < nblk - 1 else lastp
            nc.vector.tensor_tensor(out=maxacc[0:np_, :], in0=maxacc[0:np_, :],
                                    in1=x2[0:np_, :], op=OP.max)

        if stage_lim >= 4:
            phaseE(2, table2, er2_sb, bounce2, fin2)
        nc.sync.dma_start(t_omax[:], maxacc[:])
      except _StageDone:
        pass

    nc.compile()
    return nc


# =========================================================================
# entry point
# =========================================================================

_REAL_CFG = Cfg()


def _run(cfg: Cfg, inputs, trace=False):
    in_maps, meta = _host_prep(cfg, inputs)
    nc = _build(cfg, meta)
    res = run_bass_kernel_spmd(
        nc, in_maps, core_ids=list(range(cfg.n_cores)),
        trace=trace)
    omax = np.stack([r["omax"] for r in res.results])      # [cores, 128, F]
    pooled_ilv = omax.max(axis=(0, 1))
    pooled = pooled_ilv[POS_OF_NAT]                        # back to natural
    return pooled.astype(np.float32), res


def kernel(**inputs):
    cfg = _REAL_CFG
    pooled, _ = _run(cfg, inputs, trace=bool(int(os.environ.get("GAT_TRACE", "0"))))
    Ws = _f32(inputs["Ws"]); bs = _f32(inputs["bs"])
    Wc = _f32(inputs["Wc"]); bc = _f32(inputs["bc"])
    h = np.maximum(pooled @ Ws + bs, 0.0)
    out = h @ Wc + bc
    return out.reshape(1, 2).astype(np.float32)
